# revision 29
# baseline (speedup 1.0000x reference)
"""Classical self-attention (head-summed scores) on 8 trn2 NeuronCores.

Key algebraic rewrite: the reference sums scores over heads AND head dim,
so  S = (x Wq)(x Wk)^T / 8 = x A x^T  with A = Wq Wk^T / 8, and
    out = softmax(S) (x Wv) Wout + b = softmax(S) x W2 + b,  W2 = Wv Wout.
A and W2 are [E, E] weight-only products folded on the host, which removes
the K/V projections and the output projection from the device entirely.

Per-core math (core c = (batch b, query-half): 1024 queries, 2048 keys):
    T^T = A^T x_q^T            [E, 1024]   (the "query" projection)
    S^T[k, q] = x^T^T T^T      per 128-key tile, PSUM f32
    P = exp(S^T)               bf16, no max-subtraction (scores ~ N(0,4))
    U^T = sum_m x_m^T P^T[m]   [E, 1024]   (P x, contracted over keys)
    y = (U W2) * recip + b     natural layout, rowsums via ones-matmul

All matmuls bf16 (rate 1.0 cycles/row, same as fp32r, half the SBUF/DMA);
everything SBUF-resident — no DRAM staging round-trips.  ~393k PE rows
per core ≈ 164 us floor.

Scheduling notes:
  - T phase runs as 2 waves of 8 interleaved PSUM chains (fi outer) so PE
    consumes the (a[fi], xTq[fi]) DMA pairs as they arrive instead of
    stalling a single chain on the last pair.
  - Y phase post-processing is chunked: ACT applies the 1/rowsum scale
    straight out of PSUM (per-partition AP scale), DVE adds the bias,
    output store is bf16 — keeps the post-matmul tail short.
"""

import sys

sys.path.insert(0, "/opt/trn_rl_repo")

import numpy as np
from ml_dtypes import bfloat16

import concourse.bass as bass
import concourse.mybir as mybir
import concourse.tile as tile
from concourse import bacc

B, N, E = 4, 2048, 1024
NQ = N // 2          # query rows per core
P = 128              # partitions
FT = E // P          # 8 feature tiles
MT = N // P          # 16 key tiles
QT = NQ // P         # 8 query tiles
H = NQ // 2          # 512-column matmul halves (one PSUM bank)
F32 = mybir.dt.float32
BF16 = mybir.dt.bfloat16


def build_program():
    nc = bacc.Bacc("TRN2", target_bir_lowering=False, debug=False)
    xT = nc.dram_tensor("xT", [E, N], BF16, kind="ExternalInput").ap()
    xn = nc.dram_tensor("xn", [N, E], BF16, kind="ExternalInput").ap()
    a = nc.dram_tensor("a", [E, E], BF16, kind="ExternalInput").ap()
    w2 = nc.dram_tensor("w2", [E, E], BF16, kind="ExternalInput").ap()
    bout = nc.dram_tensor("bout", [E], BF16, kind="ExternalInput").ap()
    y = nc.dram_tensor("y", [NQ, E], BF16, kind="ExternalOutput").ap()

    with tile.TileContext(nc) as tc:
        _body(nc, tc, xT, xn, a, w2, bout, y)
    nc.compile()
    return nc


def _body(nc, tc, xT, xn, a, w2, bout, y):
    cst = tc.alloc_tile_pool(name="cst", bufs=1)
    ones = cst.tile([P, 1], BF16, name="ones", tag="ones")
    nc.vector.memset(ones, 1.0)
    warm = cst.tile([P, 256], BF16, name="warm", tag="warm")
    nc.vector.memset(warm, 0.0)
    # DMA issue order tracks first-use order: the T projection consumes
    # (a[fi], xTq[fi]) pairs in fi order, so those two queues interleave;
    # xTk feeds the back half of the S phase, xn the U phase, w2/bout the
    # Y phase.
    aL_t, aR_t, xTq_t = [], [], []
    # a is split into left/right column halves: wave 1 only touches the
    # left half (fo 0-3), so its first DMA pair is smaller and lands
    # sooner; the right halves follow before wave 2 needs them.
    for f in range(FT):
        at = cst.tile([P, H], BF16, name=f"aL{f}", tag=f"aL{f}")
        nc.gpsimd.dma_start(out=at, in_=a[f * P:(f + 1) * P, 0:H])
        aL_t.append(at)
        xq = cst.tile([P, NQ], BF16, name=f"xTq{f}", tag=f"xTq{f}")
        nc.sync.dma_start(out=xq, in_=xT[f * P:(f + 1) * P, 0:NQ])
        xTq_t.append(xq)
    for f in range(FT):
        at = cst.tile([P, H], BF16, name=f"aR{f}", tag=f"aR{f}")
        nc.gpsimd.dma_start(out=at, in_=a[f * P:(f + 1) * P, H:E])
        aR_t.append(at)
    xTk_t = []
    for f in range(FT):
        xk = cst.tile([P, NQ], BF16, name=f"xTk{f}", tag=f"xTk{f}")
        nc.sync.dma_start(out=xk, in_=xT[f * P:(f + 1) * P, NQ:N])
        xTk_t.append(xk)
    xn_t = [cst.tile([P, E], BF16, name=f"xn{m}", tag=f"xn{m}")
            for m in range(MT)]
    for m in range(MT):
        nc.gpsimd.dma_start(out=xn_t[m], in_=xn[m * P:(m + 1) * P, :])
    w2_t = [cst.tile([P, E], BF16, name=f"w2{f}", tag=f"w2{f}")
            for f in range(FT)]
    for f in range(FT):
        nc.gpsimd.dma_start(out=w2_t[f], in_=w2[f * P:(f + 1) * P, :])
    bo_b = cst.tile([P, E], BF16, name="bo_b", tag="bo_b")
    bout_bcast = bass.AP(tensor=bout.tensor, offset=0, ap=[[0, P], [1, E]])
    nc.sync.dma_start(out=bo_b, in_=bout_bcast)

    tT_p = tc.alloc_tile_pool(name="tTp", bufs=1)
    tT_t = [tT_p.tile([P, NQ], BF16, name=f"tT{f}", tag=f"tT{f}")
            for f in range(FT)]

    # Warm the PE pstate ramp (~3us of dummy matmuls) while the first input
    # DMAs land, so the real chains start at full clock.
    with tc.tile_pool(name="wps", bufs=1, space="PSUM") as wpp:
        wps = wpp.tile([P, 256], F32, name="wps", tag="wps")
        for i in range(14):
            nc.tensor.matmul(wps, warm[:, 0:P], warm, start=True, stop=True)

    # ---- T^T = A^T x_q^T: 2 waves x 8 chains, fi outer so each DMA pair
    # unblocks one matmul step of every open chain ----
    with tc.tile_pool(name="tps", bufs=8, space="PSUM") as tpp:
        # Wave 1: 8 chains interleaved fi-outer, so each (a[fi], xTq[fi])
        # DMA pair unblocks one step of every open chain while loads land.
        pss = [tpp.tile([P, H], F32, name=f"tp{c}", tag="tp")
               for c in range(8)]
        for fi in range(FT):
            for c in range(8):
                fo, h = c // 2, c % 2
                nc.tensor.matmul(
                    pss[c], aL_t[fi][:, fo * P:(fo + 1) * P],
                    xTq_t[fi][:, h * H:(h + 1) * H],
                    start=(fi == 0), stop=(fi == FT - 1))
        for c in range(8):
            fo, h = c // 2, c % 2
            nc.vector.tensor_copy(tT_t[fo][:, h * H:(h + 1) * H], pss[c])
        # Wave 2: inputs all present by now; sequential chains so the
        # PSUM->SBUF copies spread out instead of bunching before S starts.
        for c in range(8):
            fo, h = 4 + c // 2, c % 2
            ps = tpp.tile([P, H], F32, name=f"tp2{c}", tag="tp")
            for fi in range(FT):
                nc.tensor.matmul(
                    ps, aR_t[fi][:, (fo - 4) * P:(fo - 3) * P],
                    xTq_t[fi][:, h * H:(h + 1) * H],
                    start=(fi == 0), stop=(fi == FT - 1))
            nc.vector.tensor_copy(tT_t[fo][:, h * H:(h + 1) * H], ps)

    # ---- S^T per key tile; P = exp(S^T); rowsums lag one tile ----
    recp = tc.alloc_tile_pool(name="recp", bufs=1, side="right")
    pres = tc.alloc_tile_pool(name="pres", bufs=1)
    smp = tc.alloc_tile_pool(name="smp", bufs=1, side="right")
    sums_acc = smp.tile([P, QT], F32, name="sums_acc", tag="sums_acc")
    uT_p = tc.alloc_tile_pool(name="uTp", bufs=1, side="right")
    uT_t = [uT_p.tile([P, NQ], BF16, name=f"uT{f}", tag=f"uT{f}")
            for f in range(FT)]
    p_t = []
    # S and U PSUM pools share one scope (4 + 2 + 2 = 8 banks), so the U
    # chains start the moment p[0] exists instead of waiting for the whole
    # S-pool range to release.
    with tc.tile_pool(name="sps", bufs=2, space="PSUM") as spp, \
         tc.tile_pool(name="sums", bufs=2, space="PSUM") as sumsp, \
         tc.tile_pool(name="ups", bufs=2, space="PSUM") as upp:
        for m in range(MT):
            xcol = xTq_t if m < QT else xTk_t
            mm = m % QT
            s = spp.tile([P, NQ], F32, name="s", tag="s")
            for f in range(FT):
                for h in range(2):
                    nc.tensor.matmul(
                        s[:, h * H:(h + 1) * H],
                        xcol[f][:, mm * P:(mm + 1) * P],
                        tT_t[f][:, h * H:(h + 1) * H],
                        start=(f == 0), stop=(f == FT - 1))
            p = pres.tile([P, NQ], BF16, name=f"p{m}", tag=f"p{m}")
            nc.scalar.activation(p, s, mybir.ActivationFunctionType.Exp)
            p_t.append(p)
            if m > 0:
                _row_sums(nc, p_t[m - 1], sumsp, ones, sums_acc,
                          first=(m == 1))
        _row_sums(nc, p_t[MT - 1], sumsp, ones, sums_acc, first=False)
        recip = recp.tile([P, QT], F32, name="recip", tag="recip")
        nc.vector.reciprocal(recip, sums_acc)

        # ---- U^T[f] = sum_m xn[m][:, f]^T P^T[m], half-tile chains ----
        for fo in range(FT):
            for hh in range(2):
                u_ps = upp.tile([P, H], F32, name="u", tag="u")
                for m in range(MT):
                    nc.tensor.matmul(
                        u_ps,
                        xn_t[m][:, fo * P:(fo + 1) * P],
                        p_t[m][:, hh * H:(hh + 1) * H],
                        start=(m == 0), stop=(m == MT - 1))
                nc.vector.tensor_copy(uT_t[fo][:, hh * H:(hh + 1) * H], u_ps)

    # ---- y = (U W2) * recip + b, natural [q, e] layout ----
    with tc.tile_pool(name="yps", bufs=4, space="PSUM") as ypp, \
         tc.tile_pool(name="ysb", bufs=8) as ysp:
        for qt in range(QT):
            # Two separate PSUM tiles per qt: readers on different engines
            # never serialize on a shared tile.
            yps = [ypp.tile([P, H], F32, name=f"yps{h}", tag="yps")
                   for h in range(2)]
            for f in range(FT):
                for h in range(2):
                    nc.tensor.matmul(
                        yps[h],
                        uT_t[f][:, qt * P:(qt + 1) * P],
                        w2_t[f][:, h * H:(h + 1) * H],
                        start=(f == 0), stop=(f == FT - 1))
            if qt < QT - 1:
                for h in range(2):
                    ysb = ysp.tile([P, H], BF16, name="ysb", tag="ysb")
                    nc.scalar.activation(ysb, yps[h],
                                         mybir.ActivationFunctionType.Copy,
                                         scale=recip[:, qt:qt + 1])
                    nc.vector.tensor_tensor(out=ysb, in0=ysb,
                                            in1=bo_b[:, h * H:(h + 1) * H],
                                            op=mybir.AluOpType.add)
                    # h0 stores ride SWDGE so the final h1 store never
                    # queues behind another HWDGE grab.
                    eng = nc.gpsimd if h == 0 else nc.sync
                    eng.dma_start(
                        out=y[qt * P:(qt + 1) * P, h * H:(h + 1) * H],
                        in_=ysb)
            else:
                # Last tile: ACT scales chunk 1 while DVE scales chunk 0 in
                # parallel, then DVE finishes chunk 1 first so the final
                # store leaves as early as possible.
                y0 = ysp.tile([P, H], BF16, name="ysb", tag="ysb")
                y1 = ysp.tile([P, H], BF16, name="ysb", tag="ysb")
                nc.scalar.activation(y1, yps[1],
                                     mybir.ActivationFunctionType.Copy,
                                     scale=recip[:, qt:qt + 1])
                nc.vector.tensor_scalar_mul(y0, yps[0], recip[:, qt:qt + 1])
                nc.vector.tensor_tensor(out=y1, in0=y1, in1=bo_b[:, H:NQ],
                                        op=mybir.AluOpType.add)
                nc.sync.dma_start(out=y[qt * P:(qt + 1) * P, H:NQ], in_=y1)
                nc.vector.tensor_tensor(out=y0, in0=y0, in1=bo_b[:, 0:H],
                                        op=mybir.AluOpType.add)
                nc.sync.dma_start(out=y[qt * P:(qt + 1) * P, 0:H], in_=y0)

    pres.release()
    uT_p.release()
    smp.release()
    recp.release()
    tT_p.release()
    cst.release()


def _row_sums(nc, p, sumsp, ones, sums_acc, first):
    sums_m = sumsp.tile([P, QT], F32, name="sums_m", tag="sums_m")
    for q in range(QT):
        nc.tensor.matmul(sums_m[:, q:q + 1], p[:, q * P:(q + 1) * P], ones,
                         start=True, stop=True)
    if first:
        nc.vector.tensor_copy(sums_acc, sums_m)
    else:
        nc.vector.tensor_tensor(out=sums_acc, in0=sums_acc,
                                in1=sums_m, op=mybir.AluOpType.add)


_NC_CACHE = None


def _get_program():
    global _NC_CACHE
    if _NC_CACHE is None:
        _NC_CACHE = build_program()
    return _NC_CACHE


def _host_prep(x, W_qkv, W_out, b_out):
    """Fold weights and build the per-core input maps."""
    Wq = W_qkv[:, :E]
    Wk = W_qkv[:, E:2 * E]
    Wv = W_qkv[:, 2 * E:]
    A = ((Wq @ Wk.T) * 0.125).astype(bfloat16)
    W2 = (Wv @ W_out).astype(bfloat16)
    bo = b_out.astype(bfloat16)
    in_maps = []
    for c in range(8):
        b, half = divmod(c, 2)
        xb = x[b]
        # Rotate so this core's 1024 query rows come first; key order is
        # irrelevant (softmax sums over all keys).
        xrot = np.concatenate([xb[half * NQ:], xb[:half * NQ]], axis=0)
        xrot_bf = xrot.astype(bfloat16)
        in_maps.append({
            "xT": np.ascontiguousarray(xrot_bf.T),
            "xn": xrot_bf,
            "a": A,
            "w2": W2,
            "bout": bo,
        })
    return in_maps


def kernel(x, W_qkv, W_out, b_out):
    from concourse.bass_utils import run_bass_kernel_spmd

    x = np.asarray(x, dtype=np.float32)
    W_qkv = np.asarray(W_qkv, dtype=np.float32)
    W_out = np.asarray(W_out, dtype=np.float32)
    b_out = np.asarray(b_out, dtype=np.float32)

    nc = _get_program()
    in_maps = _host_prep(x, W_qkv, W_out, b_out)
    res = run_bass_kernel_spmd(nc, in_maps, list(range(8)))
    out = np.empty((B, N, E), dtype=np.float32)
    for c in range(8):
        b, half = divmod(c, 2)
        out[b, half * NQ:(half + 1) * NQ] = res.results[c]["y"].astype(
            np.float32)
    return out
